# revision 3
# baseline (speedup 1.0000x reference)
"""Trainium2 Bass kernel for CAWN2-style GNN message passing (v2).

Restructured from the 285us baseline around the measured engine model:
ACT (scalar engine) is the bottleneck at ~1 elem/cycle/lane @1.2GHz with a
per-instruction overhead, DVE (vector) a close second.  285us -> ~160us.

Key changes vs baseline:
- Gate biases folded into the matmuls via constant-1 rows of x: ts features
  46..63 have freq < 3e-7 so cos(arg) = 1 to <2e-5; their weight columns
  are summed with the gate biases into const rows.  ACT calls then need no
  per-partition bias, so r+z activations merge.
- GROUPED pipeline: r,z for a 2-tile group (1024 cols) go to a single
  4-bank PSUM tile and ONE sigmoid [128,2048] covers them; the group's
  xn/hn matmuls, s=(hn+b_hhn)*r (one batched stt), ident-matmul
  accumulation and per-tile tanh are deferred one group (software
  pipelining) so ACT stays saturated.  PSUM: rz 4 + xn 2x1 + hn 2 = 8
  banks exactly.  (Pairing the two xn banks into one tile to batch the
  tanh was tried and regressed badly - two open accumulation groups in
  one PSUM tile serialize the pipeline.)
- h' = n + z*(h - n): d = h - n on GPSIMD (off the DVE), w = z*d and
  hp = n + w on DVE at [128,2048] granularity, pairwise-tree K-sum
  (k-major column layout).  Elementwise for superblock s is spread in
  half-superblock steps across the next 4 group-blocks.
- x and h stream as one packed [128, 2, RK] fp16 tensor.
- merge/readout weights in fp16 (fp32 matmuls are 4x slower on PE).
- fp8 was evaluated and rejected: all-fp8 fails accuracy (4e-2) and is
  slower (GPSIMD fp8 elementwise + DoubleRow LDWEIGHTS overhead); the
  accurate mix (rz-only fp8, 8.8e-3) no longer pays since PE/DMA are not
  the bottleneck.
"""

import numpy as np

B = 4096
K = 32
F = 64
H = 128
N_CORES = 8
E = B // N_CORES            # events per core = 512
R = 3 * E                   # rows per core = 1536
RK = R * K                  # GRU rows per core = 49152
TR = 512                    # GRU rows per gate tile
SB = 2048                   # superblock rows
NSB = RK // SB              # superblocks = 24
GPS = SB // K               # event groups per superblock = 64
NTS = 46                    # ts feature rows kept (freq >= ~3e-7)
NCONST = 3                  # constant-1 bias rows

USE_FP8 = False             # data/weights dtype + DoubleRow for r/z gates
GROUPED = True              # batch sigmoid over 2-tile groups (4-bank rz)

_prog_cache = {}


def _build_program(num_devices=N_CORES, iters=1, use_fp8=None, grouped=None):
    from concourse import bacc, mybir
    import concourse.tile as tile

    if use_fp8 is None:
        use_fp8 = USE_FP8
    if grouped is None:
        grouped = GROUPED
    f32 = mybir.dt.float32
    f16 = mybir.dt.float16
    dt = mybir.dt.float8e4 if use_fp8 else f16

    nc = bacc.Bacc("TRN2", target_bir_lowering=False, debug=False,
                   num_devices=num_devices)

    # ---- DRAM I/O ----
    d_xh = nc.dram_tensor("xh", [128, 2, RK], dt, kind="ExternalInput")
    d_wpk = nc.dram_tensor("wpk", [128, 2, 3 * H], dt, kind="ExternalInput")
    d_bhn = nc.dram_tensor("bhn", [H, 1], f32, kind="ExternalInput")
    d_corr = nc.dram_tensor("corr", [H, R], f16, kind="ExternalInput")
    d_cinv = nc.dram_tensor("cinv", [H, R], f16, kind="ExternalInput")
    d_node = nc.dram_tensor("node", [F, R], f16, kind="ExternalInput")
    d_wouth = nc.dram_tensor("wouth", [H, F], f16, kind="ExternalInput")
    d_woutn = nc.dram_tensor("woutn", [F, F], f16, kind="ExternalInput")
    d_bout = nc.dram_tensor("bout", [F, 1], f32, kind="ExternalInput")
    d_fc1T = nc.dram_tensor("fc1T", [F, 2 * F], f16, kind="ExternalInput")
    d_fc1b = nc.dram_tensor("fc1b", [F, 1], f32, kind="ExternalInput")
    d_fc2T = nc.dram_tensor("fc2T", [F, 1], f16, kind="ExternalInput")
    d_fc2b = nc.dram_tensor("fc2b", [1, 1], f32, kind="ExternalInput")
    d_ident = nc.dram_tensor("ident", [128, 128], f16, kind="ExternalInput")
    d_out = nc.dram_tensor("out", [2, E], f32, kind="ExternalOutput")

    AF = mybir.ActivationFunctionType
    OP = mybir.AluOpType
    PM = mybir.MatmulPerfMode

    with tile.TileContext(nc) as tc:
        with (
            tc.tile_pool(name="const", bufs=1) as cpool,
            tc.tile_pool(name="persist", bufs=1) as ppool,
            tc.tile_pool(name="hx", bufs=3) as hxpool,
            tc.tile_pool(name="work", bufs=3 if grouped else 2) as wpool,
            tc.tile_pool(name="sub", bufs=3) as spool,
            tc.tile_pool(name="psA", bufs=1 if grouped else 2,
                         space="PSUM") as psA,
            tc.tile_pool(name="psB", bufs=2, space="PSUM") as psB,
            tc.tile_pool(name="psC", bufs=1 if grouped else 2,
                         space="PSUM") as psC,
        ):
            # ---- constants/weights ----
            wpk = cpool.tile([128, 2, 3 * H], dt, tag="wpk")
            bhn = cpool.tile([H, 1], f32, tag="bhn")
            ident = cpool.tile([128, 128], f16, tag="ident")
            wouth = cpool.tile([H, F], f16, tag="wouth")
            corr = cpool.tile([H, R], f16, tag="corr")
            cinv = cpool.tile([H, R], f16, tag="cinv")
            woutn = cpool.tile([F, F], f16, tag="woutn")
            bout = cpool.tile([F, 1], f32, tag="bout")
            fc1T = cpool.tile([F, 2 * F], f16, tag="fc1T")
            fc1b = cpool.tile([F, 1], f32, tag="fc1b")
            fc2T = cpool.tile([F, 1], f16, tag="fc2T")
            fc2b = cpool.tile([1, 1], f32, tag="fc2b")
            for t, d in [(ident, d_ident), (wpk, d_wpk), (bhn, d_bhn)]:
                nc.sync.dma_start(out=t[:], in_=d.ap())
            late_consts = [(wouth, d_wouth), (woutn, d_woutn),
                           (bout, d_bout), (fc1T, d_fc1T), (fc1b, d_fc1b),
                           (fc2T, d_fc2T), (fc2b, d_fc2b),
                           (corr, d_corr), (cinv, d_cinv)]

            XNW = TR
            # preload the sigmoid/tanh ACT table set during the first DMAs,
            # and spin the PE p-state ramp with dummy matmuls
            warm = psB.tile([H, XNW], f32, tag="xn")
            scratch = spool.tile([1, 8], f16, tag="scr")
            for i in range(16):
                nc.tensor.matmul(out=warm[:, 0:128], lhsT=ident[:],
                                 rhs=ident[:], start=(i == 0),
                                 stop=(i == 15))
                if i == 0:
                    nc.scalar.activation(out=scratch[:], in_=warm[0:1, 0:8],
                                         func=AF.Sigmoid)

            agg_all = ppool.tile([H, R], f16, tag="agg")
            agg3 = ppool.tile([H, R], f16, tag="agg3")
            node_all = ppool.tile([F, R], f16, tag="node")
            emb_all = ppool.tile([F, R], f16, tag="emb")
            pos_sb = ppool.tile([1, E], f32, tag="out0")
            neg_sb = ppool.tile([1, E], f32, tag="out1")

            # pairwise-sum cur [H, W] (k-major: col k*GPS+g) down to the
            # per-group sums agg_all[:, es*GPS : es*GPS+GPS]
            def tree_to_agg(cur, W, es):
                w = W // 2
                while True:
                    if w == GPS:
                        dst = agg_all[:, es * GPS:(es + 1) * GPS]
                        nxt = None
                    else:
                        nxt = spool.tile([H, w], f16, tag=f"tr{w}",
                                         name=f"tr{w}")
                        dst = nxt[:]
                    nc.vector.tensor_tensor(out=dst, in0=cur[:, 0:w],
                                            in1=cur[:, w:2 * w], op=OP.add)
                    if w == GPS:
                        break
                    cur, w = nxt, w // 2

            # role chunk c (src/tgt/bad): agg' = (sum-corr)*cinv, readout
            # emb = relu(W_out@[node;agg']+b), then merge scores
            def chunk_post(c, lo=0, hi=E):
                w = hi - lo
                ce = slice(c * E + lo, c * E + hi)
                agg2 = spool.tile([H, E], f16, tag="agg2")
                nc.vector.tensor_tensor(out=agg2[:, 0:w], in0=agg_all[:, ce],
                                        in1=corr[:, ce], op=OP.subtract)
                nc.vector.tensor_tensor(out=agg3[:, ce], in0=agg2[:, 0:w],
                                        in1=cinv[:, ce], op=OP.mult)
                ps_e = psB.tile([H, XNW], f32, tag="xn")
                nc.tensor.matmul(out=ps_e[0:F, 0:w], lhsT=wouth[:],
                                 rhs=agg3[:, ce], start=True, stop=False)
                nc.tensor.matmul(out=ps_e[0:F, 0:w], lhsT=woutn[:],
                                 rhs=node_all[:, ce], start=False, stop=True)
                nc.scalar.activation(out=emb_all[:, ce], in_=ps_e[0:F, 0:w],
                                     func=AF.Relu, bias=bout[:, 0:1])
                if c == 0:
                    return
                e0 = slice(lo, hi)
                ps_h1 = psB.tile([H, XNW], f32, tag="xn")
                nc.tensor.matmul(out=ps_h1[0:F, 0:w], lhsT=fc1T[:, 0:F],
                                 rhs=emb_all[:, e0], start=True, stop=False)
                nc.tensor.matmul(out=ps_h1[0:F, 0:w], lhsT=fc1T[:, F:2 * F],
                                 rhs=emb_all[:, ce],
                                 start=False, stop=True)
                h1_sb = spool.tile([F, E], f16, tag="h1_sb")
                nc.scalar.activation(out=h1_sb[:, 0:w], in_=ps_h1[0:F, 0:w],
                                     func=AF.Relu, bias=fc1b[:, 0:1])
                ps_p = psB.tile([H, XNW], f32, tag="xn")
                nc.tensor.matmul(out=ps_p[0:1, 0:w], lhsT=fc2T[:],
                                 rhs=h1_sb[:, 0:w], start=True, stop=True)
                out_t = pos_sb if c == 1 else neg_sb
                nc.scalar.activation(out=out_t[:, e0], in_=ps_p[0:1, 0:w],
                                     func=AF.Identity, bias=fc2b[:, 0:1])

            # staged elementwise for superblock ctx = (xh_sb, rz_sb, n_sb, s)
            # stage 0: d = h - n      (GPSIMD)
            # stage 1: w = z * d      (DVE)
            # stage 2: hp = n + w     (DVE)
            # stage 3: tree(hp)       (DVE)
            def emit_elem(ctx, stage, lo=0, hi=SB):
                xh_sb, rz_sb, n_sb, es, tl = ctx
                cols = slice(lo, hi)
                if stage == 0:
                    if "d" not in tl:
                        tl["d"] = wpool.tile([H, SB], f16, tag="d_sb",
                                             name="d_sb")
                    nc.gpsimd.tensor_tensor(
                        out=tl["d"][:, cols], in0=xh_sb[:, 1, cols],
                        in1=n_sb[:, cols], op=OP.subtract)
                elif stage == 1:
                    if "w" not in tl:
                        tl["w"] = wpool.tile([H, SB], f16, tag="w_sb",
                                             name="w_sb")
                    nc.vector.tensor_tensor(
                        out=tl["w"][:, cols], in0=rz_sb[:, 1, cols],
                        in1=tl["d"][:, cols], op=OP.mult)
                elif stage == 2:
                    if "hp" not in tl:
                        tl["hp"] = wpool.tile([H, SB], f16, tag="hp_sb",
                                              name="hp_sb")
                    nc.vector.tensor_tensor(
                        out=tl["hp"][:, cols], in0=n_sb[:, cols],
                        in1=tl["w"][:, cols], op=OP.add)
                else:
                    tree_to_agg(tl["hp"], SB, es)

            # deferred post-sigmoid work for a 2-tile group:
            # xn/hn matmuls, s = (hn+b)*r, ident-accumulate, one tanh
            def flush_group(pg):
                xh_p, rz_p, n_p, g0p = pg
                ps_xns = []
                for t in range(2):
                    ct = slice(g0p + t * TR, g0p + (t + 1) * TR)
                    ps_xn = psB.tile([H, XNW], f32, tag="xn")
                    nc.tensor.matmul(out=ps_xn[:],
                                     lhsT=wpk[:, 0, 2 * H:],
                                     rhs=xh_p[:, 0, ct],
                                     start=True, stop=False)
                    ps_xns.append(ps_xn)
                ps_hn = psC.tile([H, 2 * TR], f32, tag="hn")
                for t in range(2):
                    ct = slice(g0p + t * TR, g0p + (t + 1) * TR)
                    nc.tensor.matmul(out=ps_hn[:, t * TR:(t + 1) * TR],
                                     lhsT=wpk[:, 1, 2 * H:],
                                     rhs=xh_p[:, 1, ct],
                                     start=True, stop=True)
                s_g = spool.tile([H, 2 * TR], f16, tag="s_t")
                nc.vector.scalar_tensor_tensor(
                    out=s_g[:], in0=ps_hn[:], scalar=bhn[:, 0:1],
                    in1=rz_p[:, 0, g0p:g0p + 2 * TR],
                    op0=OP.add, op1=OP.mult)
                for t in range(2):
                    nc.tensor.matmul(out=ps_xns[t][:], lhsT=ident[:],
                                     rhs=s_g[:, t * TR:(t + 1) * TR],
                                     start=False, stop=True)
                for t in range(2):
                    ct = slice(g0p + t * TR, g0p + (t + 1) * TR)
                    nc.scalar.activation(out=n_p[:, ct], in_=ps_xns[t][:],
                                         func=AF.Tanh)

            def emit_grouped_body(it):
                SPC = NSB // 3
                HS = SB // 2
                pend_g = None
                ctxs = {}

                def step_0a(c):
                    emit_elem(c, 0, 0, HS)

                def step_0b1a(c):
                    emit_elem(c, 0, HS, SB)
                    emit_elem(c, 1, 0, HS)

                def step_1b2a(c):
                    emit_elem(c, 1, HS, SB)
                    emit_elem(c, 2, 0, HS)

                def step_2b3(c):
                    emit_elem(c, 2, HS, SB)
                    emit_elem(c, 3)

                for s in range(NSB):
                    j0 = s * SB
                    xh_sb = hxpool.tile([128, 2, SB], dt, tag="xh")
                    nc.sync.dma_start(out=xh_sb[:],
                                      in_=d_xh.ap()[:, :, j0:j0 + SB])
                    n_sb = wpool.tile([H, SB], f16, tag="n_sb")
                    rz_sb = wpool.tile([H, 2, SB], f16, tag="rz_sb")
                    ctxs[s] = (xh_sb, rz_sb, n_sb, s, {})
                    for gl in range(2):
                        g0 = gl * 2 * TR
                        ps_rz = psA.tile([H, 4 * TR], f32, tag="rz")
                        for t in range(2):
                            ct = slice(g0 + t * TR, g0 + (t + 1) * TR)
                            xs = xh_sb[:, 0, ct]
                            hs = xh_sb[:, 1, ct]
                            rsl = slice(t * TR, (t + 1) * TR)
                            zsl = slice(2 * TR + t * TR, 2 * TR + (t + 1) * TR)
                            nc.tensor.matmul(out=ps_rz[:, rsl],
                                             lhsT=wpk[:, 0, 0:H], rhs=xs,
                                             start=True, stop=False)
                            nc.tensor.matmul(out=ps_rz[:, rsl],
                                             lhsT=wpk[:, 1, 0:H], rhs=hs,
                                             start=False, stop=True)
                            nc.tensor.matmul(out=ps_rz[:, zsl],
                                             lhsT=wpk[:, 0, H:2 * H], rhs=xs,
                                             start=True, stop=False)
                            nc.tensor.matmul(out=ps_rz[:, zsl],
                                             lhsT=wpk[:, 1, H:2 * H], rhs=hs,
                                             start=False, stop=True)
                        nc.scalar.activation(out=rz_sb[:, :, g0:g0 + 2 * TR],
                                             in_=ps_rz[:, 0:4 * TR],
                                             func=AF.Sigmoid)
                        if pend_g is not None:
                            flush_group(pend_g)
                        pend_g = (xh_sb, rz_sb, n_sb, g0)
                        if gl == 0:
                            if s >= 1:
                                step_0b1a(ctxs[s - 1])
                            if s >= 2:
                                step_2b3(ctxs[s - 2])
                                del ctxs[s - 2]
                        else:
                            step_0a(ctxs[s])
                            if s >= 1:
                                step_1b2a(ctxs[s - 1])
                            if s == 9:
                                chunk_post(0)
                            elif s == 17:
                                chunk_post(1)
                            elif s == 21:
                                chunk_post(2, 0, E // 2)
                            elif s == 23:
                                chunk_post(2, E // 2, 3 * E // 4)
                    if s == 1 and it == 0:
                        for t, d in late_consts:
                            nc.sync.dma_start(out=t[:], in_=d.ap())
                        nc.sync.dma_start(out=node_all[:], in_=d_node.ap())

                # tail
                flush_group(pend_g)
                step_0b1a(ctxs[NSB - 1])
                step_2b3(ctxs[NSB - 2])
                step_1b2a(ctxs[NSB - 1])
                step_2b3(ctxs[NSB - 1])
                chunk_post(2, 3 * E // 4, E)

            # timing builds repeat the whole body; iters=1 for real use
            for it in range(iters):
                if grouped:
                    emit_grouped_body(it)
                    nc.sync.dma_start(out=d_out.ap()[0:1, :], in_=pos_sb[:])
                    nc.sync.dma_start(out=d_out.ap()[1:2, :], in_=neg_sb[:])
                    continue
                SPC = NSB // 3       # superblocks per role chunk
                pend = None          # (ps_xn, s_t, n_dest) awaiting ident+tanh
                pend_elem = None     # elem ctx for SB s-1
                for s in range(NSB):
                    j0 = s * SB
                    last = s == NSB - 1

                    xh_sb = hxpool.tile([128, 2, SB], dt, tag="xh")
                    nc.sync.dma_start(out=xh_sb[:],
                                      in_=d_xh.ap()[:, :, j0:j0 + SB])
                    n_sb = wpool.tile([H, SB], f16, tag="n_sb")
                    rz_sb = wpool.tile([H, 2, SB], f16, tag="rz_sb")

                    for t4 in range(SB // TR):
                        a0 = t4 * TR
                        cs = slice(a0, a0 + TR)
                        xs = xh_sb[:, 0, cs]
                        hs = xh_sb[:, 1, cs]
                        ps_rz = psA.tile([H, 2 * TR], f32, tag="rz")
                        if use_fp8:
                            nc.tensor.matmul(out=ps_rz[:, 0:TR],
                                             lhsT=wpk[:, :, 0:H],
                                             rhs=xh_sb[:, :, cs],
                                             start=True, stop=True,
                                             perf_mode=PM.DoubleRow)
                            nc.tensor.matmul(out=ps_rz[:, TR:2 * TR],
                                             lhsT=wpk[:, :, H:2 * H],
                                             rhs=xh_sb[:, :, cs],
                                             start=True, stop=True,
                                             perf_mode=PM.DoubleRow)
                        else:
                            nc.tensor.matmul(out=ps_rz[:, 0:TR],
                                             lhsT=wpk[:, 0, 0:H],
                                             rhs=xs, start=True, stop=False)
                            nc.tensor.matmul(out=ps_rz[:, 0:TR],
                                             lhsT=wpk[:, 1, 0:H],
                                             rhs=hs, start=False, stop=True)
                            nc.tensor.matmul(out=ps_rz[:, TR:2 * TR],
                                             lhsT=wpk[:, 0, H:2 * H],
                                             rhs=xs, start=True, stop=False)
                            nc.tensor.matmul(out=ps_rz[:, TR:2 * TR],
                                             lhsT=wpk[:, 1, H:2 * H],
                                             rhs=hs, start=False, stop=True)
                        ps_xn = psB.tile([H, XNW], f32, tag="xn")
                        nc.tensor.matmul(out=ps_xn[:], lhsT=wpk[:, 0, 2 * H:],
                                         rhs=xs, start=True, stop=False)
                        ps_hn = psC.tile([H, TR], f32, tag="hn")
                        nc.tensor.matmul(out=ps_hn[:], lhsT=wpk[:, 1, 2 * H:],
                                         rhs=hs, start=True, stop=True)

                        # r,z in one sigmoid (biases folded into matmul)
                        nc.scalar.activation(out=rz_sb[:, :, cs],
                                             in_=ps_rz[:, 0:2 * TR],
                                             func=AF.Sigmoid)
                        s_t = spool.tile([H, TR], f16, tag="s_t")
                        nc.vector.scalar_tensor_tensor(
                            out=s_t[:], in0=ps_hn[:], scalar=bhn[:, 0:1],
                            in1=rz_sb[:, 0, cs], op0=OP.add, op1=OP.mult)

                        if pend is not None:
                            p_xn, p_s, p_dst = pend
                            nc.tensor.matmul(out=p_xn[:], lhsT=ident[:],
                                             rhs=p_s[:], start=False,
                                             stop=True)
                            nc.scalar.activation(out=p_dst, in_=p_xn[:],
                                                 func=AF.Tanh)
                        pend = (ps_xn, s_t, n_sb[:, cs])
                        if last and t4 == 3:
                            p_xn, p_s, p_dst = pend
                            nc.tensor.matmul(out=p_xn[:], lhsT=ident[:],
                                             rhs=p_s[:], start=False,
                                             stop=True)
                            nc.scalar.activation(out=p_dst, in_=p_xn[:],
                                                 func=AF.Tanh)
                            pend = None

                        if pend_elem is not None and t4 >= 1:
                            emit_elem(pend_elem, t4 - 1)
                            if t4 == 3:
                                emit_elem(pend_elem, 3)
                                pend_elem = None

                    if not last:
                        pend_elem = (xh_sb, rz_sb, n_sb, s, {})
                        if s % SPC == 0 and s > 0:
                            chunk_post(s // SPC - 1)
                        if s == 2 * SPC + SPC // 2:
                            chunk_post(2, 0, E // 2)
                        if s == 1 and it == 0:
                            for t, d in late_consts:
                                nc.sync.dma_start(out=t[:], in_=d.ap())
                            nc.sync.dma_start(out=node_all[:], in_=d_node.ap())
                    else:
                        # drain: elem for the final superblock
                        ctx = (xh_sb, rz_sb, n_sb, s, {})
                        for stg in range(4):
                            emit_elem(ctx, stg)
                        chunk_post(2, E // 2, E)

                nc.sync.dma_start(out=d_out.ap()[0:1, :], in_=pos_sb[:])
                nc.sync.dma_start(out=d_out.ap()[1:2, :], in_=neg_sb[:])

    nc.compile()
    return nc


def _prep_inputs(inputs, use_fp8=None):
    """Host-side staging: gathers, time encoding, packing, constant folds."""
    if use_fp8 is None:
        use_fp8 = USE_FP8
    import ml_dtypes
    np_dt = ml_dtypes.float8_e4m3 if use_fp8 else np.float16

    f = lambda k: np.asarray(inputs[k], dtype=np.float32)
    ii = lambda k: np.asarray(inputs[k], dtype=np.int64)

    src, tgt, bad = ii("src_ids"), ii("tgt_ids"), ii("bad_ids")
    cut = f("cut_time")
    ngh_id, e_idx, ngh_ts = ii("ngh_id"), ii("e_idx"), f("ngh_ts")
    hidden = f("hidden_store")
    n_feat, e_feat = f("n_feat"), f("e_feat")
    basis_freq, phase = f("basis_freq"), f("phase")
    W_ih, W_hh = f("W_ih"), f("W_hh")
    b_ih, b_hh = f("b_ih"), f("b_hh")
    W_out, b_out = f("W_out"), f("b_out")
    fc1_w, fc1_b = f("fc1_w"), f("fc1_b")
    fc2_w, fc2_b = f("fc2_w"), f("fc2_b")

    wihT = np.ascontiguousarray(W_ih.T)                       # [DIN, 3H] f32
    whhT = np.ascontiguousarray(W_hh.T)                       # [H, 3H] f32
    # fold rows F+NTS..F+63 (cos(arg)~1) plus gate biases into const rows.
    # biases: r,z get b_ih+b_hh; n gets b_ih only (b_hh,n applied in stt).
    fold = wihT[F + NTS:F + 64, :].sum(axis=0)                # [3H]
    fold = fold + np.concatenate([b_ih[0:H] + b_hh[0:H],
                                  b_ih[H:2 * H] + b_hh[H:2 * H],
                                  b_ih[2 * H:3 * H]])
    # x-half rows: 0..63 e_emb, 64..64+NTS-1 ts, then NCONST const rows
    # carrying hi/lo/lolo splits of fold, rest zero.
    xw = np.zeros((128, 3 * H), dtype=np.float32)
    xw[0:F, :] = wihT[0:F, :]
    xw[F:F + NTS, :] = wihT[F:F + NTS, :]
    rem = fold.copy()
    for j in range(NCONST):
        q = rem.astype(np_dt).astype(np.float32)
        xw[F + NTS + j, :] = q
        rem -= q
    wpk = np.zeros((128, 2, 3 * H), dtype=np_dt)
    wpk[:, 0, :] = xw.astype(np_dt)
    wpk[:, 1, :] = whhT.astype(np_dt)

    bhn = b_hh[2 * H:3 * H].reshape(H, 1).astype(np.float32)

    woutT = W_out.T                                           # [F+H, F]
    woutn = np.ascontiguousarray(woutT[0:F, :]).astype(np.float16)
    wouth = np.ascontiguousarray(woutT[F:F + H, :]).astype(np.float16)
    boutc = b_out.reshape(F, 1).astype(np.float32)
    fc1Tfull = fc1_w.T                                        # [2F, F]
    fc1T = np.concatenate([fc1Tfull[0:F, :], fc1Tfull[F:2 * F, :]],
                          axis=1).astype(np.float16)          # [F, 2F]
    fc1bc = fc1_b.reshape(F, 1).astype(np.float32)
    fc2T = np.ascontiguousarray(fc2_w.T).astype(np.float16)   # [F, 1]
    fc2bc = fc2_b.reshape(1, 1).astype(np.float32)
    ident = np.eye(128, dtype=np.float16)

    shared = dict(wpk=wpk, bhn=bhn, wouth=wouth, woutn=woutn, bout=boutc,
                  fc1T=fc1T, fc1b=fc1bc, fc2T=fc2T, fc2b=fc2bc, ident=ident)

    n_feat16 = n_feat.astype(np.float16)
    e_feat_dt = e_feat.astype(np_dt)

    # GRU output for an all-zero (x=h=0) column, mimicking the device's
    # op order and fp16 rounding:
    #   r=z=sigmoid(0)=0.5, s=f16((0+bhn)*0.5), n=f16(tanh(s)),
    #   d=f16(0-n), w=f16(0.5*d), hp=f16(n+w)
    bhn1 = b_hh[2 * H:3 * H].astype(np.float32)
    s0 = (bhn1 * 0.5).astype(np.float16).astype(np.float32)
    n0 = np.tanh(s0).astype(np.float16).astype(np.float32)
    d0 = (-n0).astype(np.float16).astype(np.float32)
    w0 = (0.5 * d0).astype(np.float16).astype(np.float32)
    c0 = (n0 + w0).astype(np.float16)                         # [H]

    in_maps = []
    for c in range(N_CORES):
        sl = slice(c * E, (c + 1) * E)
        rows = np.concatenate([np.arange(sl.start, sl.stop),
                               B + np.arange(sl.start, sl.stop),
                               2 * B + np.arange(sl.start, sl.stop)])
        ids_c = np.concatenate([src[sl], tgt[sl], bad[sl]])
        ct3 = np.concatenate([cut[sl]] * 3)                   # [R]
        dt_c = (ct3[:, None] - ngh_ts[rows]).astype(np.float32)   # [R,K]
        # ts features for rows 0..NTS-1 (host cos in f64, matching the
        # reference's rounding), feature-major
        arg32 = (basis_freq[0:NTS, None, None] * dt_c[None, :, :]).astype(
            np.float32)                                        # [NTS,R,K]
        a64 = (arg32.astype(np.float64)
               + phase[0:NTS].astype(np.float64)[:, None, None])
        kmaj = lambda a: np.ascontiguousarray(
            a.reshape(a.shape[0], NSB, GPS, K).transpose(0, 1, 3, 2)
            .reshape(a.shape[0], RK))
        ts3 = np.cos(a64).astype(np_dt)                        # [NTS,R,K]
        eg3 = np.ascontiguousarray(
            e_feat_dt[e_idx[rows]].transpose(2, 0, 1))         # [F,R,K]
        h3 = hidden[rows].astype(np_dt).transpose(2, 0, 1)     # [H,R,K]
        mask2 = ngh_id[rows] == 0                              # [R,K]
        if mask2.any():
            ts3[:, mask2] = 0
            eg3[:, mask2] = 0
            h3[:, mask2] = 0
        xh_c = np.zeros((128, 2, RK), dtype=np_dt)
        xh_c[0:F, 0, :] = kmaj(eg3)
        xh_c[F:F + NTS, 0, :] = kmaj(ts3)
        one_col = np.ones((RK,), dtype=np_dt)
        if mask2.any():
            mk = kmaj(mask2[None].astype(np.float32))[0] > 0.5
            one_col[mk] = 0
        for j in range(NCONST):
            xh_c[F + NTS + j, 0, :] = one_col
        xh_c[:, 1, :] = kmaj(h3)
        cnt = (K - mask2.sum(1)).astype(np.float32)            # [R]
        inv = (1.0 / np.maximum(cnt, 1.0)).astype(np.float16)
        corr_c = np.ascontiguousarray(
            c0[:, None].astype(np.float32)
            * (K - cnt)[None, :]).astype(np.float16)           # [H,R]
        cinv_c = np.ascontiguousarray(
            np.broadcast_to(inv[None, :], (H, R)))
        node_c = np.ascontiguousarray(n_feat16[ids_c].T)       # [F,R]
        m = dict(shared)
        m.update(xh=xh_c, corr=corr_c, cinv=cinv_c, node=node_c)
        in_maps.append(m)
    return in_maps


def kernel(**inputs) -> np.ndarray:
    from concourse.bass_utils import run_bass_kernel_spmd

    key = f"nc{int(USE_FP8)}"
    if key not in _prog_cache:
        _prog_cache[key] = _build_program()
    nc = _prog_cache[key]

    in_maps = _prep_inputs(inputs)
    res = run_bass_kernel_spmd(nc, in_maps, list(range(N_CORES)))

    out = np.empty((B, 2), dtype=np.float32)
    for c in range(N_CORES):
        o = res.results[c]["out"]                             # [2, E]
        out[c * E:(c + 1) * E, 0] = o[0]
        out[c * E:(c + 1) * E, 1] = o[1]
    return out
